# revision 7
# baseline (speedup 1.0000x reference)
"""Trainium2 Bass kernel for a KG decoder: scores = (sbj @ W_r[rel]) . obj.

Shapes (fixed): sbj_embs [1024,1,512] f32, obj_embs [1024,64,512] f32,
rel_ids [1024] int, W_r [200,512,512] f32 -> scores [1024,64] f32.

Strategy: sort the batch by rel_id on the host and give each of the 8 cores a
contiguous 128-element chunk plus the compacted slice of W_r its chunk needs
(~29 matrices instead of a 128-matrix gather). On device, a one-hot mask per
relation slot zeroes the subject columns that don't belong to that relation,
so v[b] = sbj[b] @ W[rel_b] falls out of a single PSUM accumulation chain
over all relation slots. Scores are a fused multiply-reduce of v against obj.
"""

import numpy as np

D = 512          # embedding dim
NOBJ = 64        # candidate objects per example
B = 1024         # batch
BC = 128         # batch per core
NCORES = 8
KCH = 4          # 512 = 4 chunks of 128 along the contraction dim
P = 128

PROFILE = False          # test.py sets True to collect an NTFF trace
LAST_RESULT = None       # BassKernelResults of the last run (for profiling)
LAST_IN_MAPS = None      # per-core input maps of the last run (for timing)

_COMPILED = {}


def _build(r_max):
    import concourse.bacc as bacc
    import concourse.mybir as mybir
    import concourse.tile as tile

    f32 = mybir.dt.float32
    mult = mybir.AluOpType.mult
    add = mybir.AluOpType.add

    nc = bacc.Bacc(
        "TRN2", target_bir_lowering=False, debug=False, num_devices=NCORES
    )
    sbjT = nc.dram_tensor("sbjT", [D, BC], f32, kind="ExternalInput").ap()
    obj = nc.dram_tensor("obj", [BC, NOBJ * D], f32, kind="ExternalInput").ap()
    wsl = nc.dram_tensor("wsl", [r_max, D, D], f32, kind="ExternalInput").ap()
    ohT = nc.dram_tensor("ohT", [1, r_max * BC], f32, kind="ExternalInput").ap()
    scores = nc.dram_tensor("scores", [BC, NOBJ], f32, kind="ExternalOutput").ap()

    with tile.TileContext(nc) as tc:
        with (
            tc.tile_pool(name="const", bufs=1) as cpool,
            tc.tile_pool(name="wpool", bufs=4) as wpool,
            tc.tile_pool(name="opool", bufs=4) as opool,
            tc.tile_pool(name="scratch", bufs=2) as spool,
            tc.tile_pool(name="psum", bufs=1, space="PSUM") as ppool,
        ):
            sbjT_t = cpool.tile([P, KCH, BC], f32)
            nc.sync.dma_start(
                out=sbjT_t[:], in_=sbjT.rearrange("(c p) b -> p c b", p=P)
            )
            # One-hot replicated across partitions via broadcast DMA:
            # oh_full[p, l*BC + b] = onehot[b, l] for every partition p.
            oh_full = cpool.tile([P, r_max * BC], f32)
            nc.sync.dma_start(
                out=oh_full[:], in_=ohT[0:1].to_broadcast([P, r_max * BC])
            )

            # Masked lhsT chunks: msk[c][d, l, b] = sbjT[c*128+d, b] * onehot[b, l]
            msk = []
            for c in range(KCH):
                m = cpool.tile([P, r_max, BC], f32, tag=f"msk{c}")
                nc.vector.tensor_tensor(
                    out=m[:],
                    in0=sbjT_t[:, c, :][:, None, :].to_broadcast([P, r_max, BC]),
                    in1=oh_full[:].rearrange("p (l b) -> p l b", b=BC),
                    op=mult,
                )
                msk.append(m)

            # v[b] = sbj[b] @ W[rel_b], via masked accumulation over slots.
            vps = ppool.tile([P, D], f32)
            for l in range(r_max):
                wt = wpool.tile([P, KCH, D], f32, tag="wt")
                nc.sync.dma_start(
                    out=wt[:], in_=wsl[l].rearrange("(c p) e -> p c e", p=P)
                )
                for c in range(KCH):
                    nc.tensor.matmul(
                        vps[:],
                        msk[c][:, l, :],
                        wt[:, c, :],
                        start=(l == 0 and c == 0),
                        stop=(l == r_max - 1 and c == KCH - 1),
                    )
            v = cpool.tile([P, D], f32)
            nc.vector.tensor_copy(out=v[:], in_=vps[:])

            # scores[b, m] = sum_e obj[b, m*512+e] * v[b, e]
            sc = cpool.tile([P, NOBJ], f32)
            MW = 8
            for mc in range(NOBJ // MW):
                ot = opool.tile([P, MW * D], f32, tag="ot")
                nc.sync.dma_start(
                    out=ot[:], in_=obj[:, mc * MW * D : (mc + 1) * MW * D]
                )
                prod = spool.tile([P, MW, D], f32, tag="prod")
                nc.vector.tensor_tensor(
                    out=prod[:],
                    in0=ot[:].rearrange("p (m e) -> p m e", e=D),
                    in1=v[:, None, :].to_broadcast([P, MW, D]),
                    op=mult,
                )
                nc.vector.reduce_sum(
                    out=sc[:, mc * MW : (mc + 1) * MW],
                    in_=prod[:],
                    axis=mybir.AxisListType.X,
                )
            nc.sync.dma_start(out=scores[:], in_=sc[:])
    if not nc.is_finalized():
        nc.finalize()
    return nc


def _get_compiled(r_max):
    if r_max not in _COMPILED:
        _COMPILED[r_max] = _build(r_max)
    return _COMPILED[r_max]


def prepare(sbj_embs, obj_embs, rel_ids, W_r):
    """Host-side sharding: sort by rel_id, compact per-core W slices."""
    sbj = np.asarray(sbj_embs, dtype=np.float32).reshape(B, D)
    obj = np.asarray(obj_embs, dtype=np.float32).reshape(B, NOBJ * D)
    rel = np.asarray(rel_ids).astype(np.int64)
    W = np.asarray(W_r, dtype=np.float32)

    order = np.argsort(rel, kind="stable")
    percore = []
    for c in range(NCORES):
        idx = order[c * BC : (c + 1) * BC]
        uniq, lidx = np.unique(rel[idx], return_inverse=True)
        percore.append((idx, uniq, lidx))
    r_max = max(len(u) for _, u, _ in percore)

    in_maps = []
    for idx, uniq, lidx in percore:
        wsl = np.zeros((r_max, D, D), np.float32)
        wsl[: len(uniq)] = W[uniq]
        ohT = np.zeros((r_max, BC), np.float32)
        ohT[lidx, np.arange(BC)] = 1.0
        in_maps.append(
            {
                "sbjT": np.ascontiguousarray(sbj[idx].T),
                "obj": np.ascontiguousarray(obj[idx]),
                "wsl": wsl,
                "ohT": ohT.reshape(1, r_max * BC),
            }
        )
    return r_max, percore, in_maps


def kernel(sbj_embs, obj_embs, rel_ids, W_r):
    global LAST_RESULT
    r_max, percore, in_maps = prepare(sbj_embs, obj_embs, rel_ids, W_r)
    nc = _get_compiled(r_max)

    from concourse.bass_utils import run_bass_kernel_spmd

    global LAST_IN_MAPS
    LAST_IN_MAPS = in_maps
    res = run_bass_kernel_spmd(
        nc, in_maps, core_ids=list(range(NCORES)), trace=PROFILE
    )
    LAST_RESULT = res

    out = np.empty((B, NOBJ), np.float32)
    for c in range(NCORES):
        out[percore[c][0]] = res.results[c]["scores"]
    return out
